# revision 1
# baseline (speedup 1.0000x reference)
"""Trainium2 Bass kernel for nn_DWTModelSimple.

The reference computes a 2-level orthonormal Haar DWT and immediately
inverts it with the exact same cached high-frequency subbands:
idwt(idwt(dwt(dwt(x)))) == x exactly (the transform is its own inverse
with all subbands kept; per 2x2 block the butterfly reconstructs a,b,c,d
exactly).  The whole module is the identity map on x; the only
deviation the float32 reference shows from x is its own rounding noise
(~6e-8 norm-relative).  The memory-roofline implementation is therefore
a straight HBM->HBM copy, data-parallel over the batch dimension.

Sharding: batch 32 -> 4 per core across 8 NeuronCores.  Each core copies
its contiguous 4*3*512*512 fp32 slice (12.58 MB) from the input DRAM
tensor to the output DRAM tensor with large HWDGE DMA transfers spread
over both hardware descriptor-generation rings (SP + ACT).
"""

import numpy as np

import concourse.bass as bass
import concourse.mybir as mybir
from concourse.bass_utils import run_bass_kernel_spmd

N_CORES = 8
B, C, H, W = 32, 3, 512, 512
B_PER_CORE = B // N_CORES
ELEMS_PER_CORE = B_PER_CORE * C * H * W  # 3,145,728
P = 128
FREE = ELEMS_PER_CORE // P  # 24576

# DMA chunking: rows of the [128, 24576] per-core view per transfer.
# 8 chunks of 16 rows = 1.57 MB each, alternating between the two HWDGE
# rings (sync=SP, scalar=ACT).
N_CHUNKS = 8
ROWS_PER_CHUNK = P // N_CHUNKS

_cached_nc = None


def _build_nc() -> bass.Bass:
    nc = bass.Bass()
    x = nc.dram_tensor("x", [P, FREE], mybir.dt.float32, kind="ExternalInput")
    y = nc.dram_tensor("y", [P, FREE], mybir.dt.float32, kind="ExternalOutput")

    with (
        nc.semaphore("sem_sp") as sem_sp,
        nc.semaphore("sem_act") as sem_act,
        nc.Block() as block,
    ):
        n_sp = 0
        n_act = 0
        chunks = [
            (y[i * ROWS_PER_CHUNK : (i + 1) * ROWS_PER_CHUNK, :],
             x[i * ROWS_PER_CHUNK : (i + 1) * ROWS_PER_CHUNK, :])
            for i in range(N_CHUNKS)
        ]
        for i in range(0, N_CHUNKS, 2):
            n_sp += 1
            n_act += 1

        @block.sync
        def _(sync):
            cnt = 0
            for i in range(0, N_CHUNKS, 2):
                dst, src = chunks[i]
                sync.dma_start(dst, src).then_inc(sem_sp, 16)
                cnt += 1
            sync.wait_ge(sem_sp, 16 * cnt)

        @block.scalar
        def _(scalar):
            cnt = 0
            for i in range(1, N_CHUNKS, 2):
                dst, src = chunks[i]
                scalar.dma_start(dst, src).then_inc(sem_act, 16)
                cnt += 1
            scalar.wait_ge(sem_act, 16 * cnt)

    return nc


def get_nc() -> bass.Bass:
    global _cached_nc
    if _cached_nc is None:
        _cached_nc = _build_nc()
    return _cached_nc


def kernel(x: np.ndarray) -> np.ndarray:
    x = np.ascontiguousarray(x, dtype=np.float32)
    assert x.shape == (B, C, H, W), x.shape

    in_maps = [
        {"x": x[i * B_PER_CORE : (i + 1) * B_PER_CORE].reshape(P, FREE)}
        for i in range(N_CORES)
    ]
    res = run_bass_kernel_spmd(get_nc(), in_maps, core_ids=list(range(N_CORES)))
    out = np.concatenate(
        [res.results[i]["y"].reshape(B_PER_CORE, C, H, W) for i in range(N_CORES)],
        axis=0,
    )
    return out
